# revision 1
# baseline (speedup 1.0000x reference)
import sys
import os

sys.path.insert(0, "/opt/trn_rl_repo")

import numpy as np

import concourse.bass as bass
import concourse.tile as tile
from concourse import mybir
from concourse.bass_utils import run_bass_kernel_spmd

# ---------------- problem constants (hardcoded) ----------------
B, N, DIM, H, DH, K = 2, 2048, 1024, 8, 64, 32
INNER = H * DH          # 512
NH = 2                  # heads per core
NT = N // 128           # 16 query/key tiles
KJD = K * DH            # 2048 floats of mem keys (or values) per query
SCALE = DH ** -0.5
NEG = -3.0e38

FP32 = mybir.dt.float32


# ---------------- drain workaround (this walrus rejects multi-wait Drain) ---
def _patched_drain(self, tick_clock, wait_clock):
    nc = self.nc
    drain_inst = nc.sync.drain()
    from concourse.tile import ScopedClock

    wait_clock.add_sem_waits(
        drain_inst.ins, ScopedClock({None: tick_clock.global_clock})
    )
    si = drain_inst.ins.sync_info
    waits = list(si.on_wait)
    if len(waits) > 1:
        drain_inst.ins.sync_info = type(si)(on_wait=waits[:1], on_update=[])
        for w in waits[1:]:
            nop = nc.sync.nop(nofuse=True)
            nop.ins.sync_info = type(si)(on_wait=[w], on_update=[])
    nc.all_engine_barrier()
    popped = nc._tile_sem_poison_stack.pop()
    assert popped is self._sem_poison
    nc.clear_and_free_semaphores(list(self.sems.allocated().values()))
    nc.all_engine_barrier()


tile.TileContext._drain_and_barrier = _patched_drain


# ---- split multi-wait instructions (walrus wait-slot limit) ----
_MAXW = 1
_orig_lower_ordered = tile.TileContext._lower_ordered_insts


def _split_lower(self, ordered):
    n = [0]
    for bbname in list(ordered.keys()):
        insts = ordered[bbname]
        new = []
        for inst in insts:
            try:
                si = inst.sync_info
                waits = list(si.on_wait) if si is not None else []
            except AttributeError:
                waits = []
            if len(waits) > _MAXW:
                keep = waits[-_MAXW:]
                extra = waits[:-_MAXW]
                for i in range(0, len(extra), _MAXW):
                    chunk = extra[i : i + _MAXW]
                    n[0] += 1
                    nop = mybir.InstNoOp(
                        name=f"waitnop-{n[0]}-{inst.name}",
                        sync_info=mybir.SyncInfo(on_wait=chunk, on_update=[]),
                        bass_nofuse=True,
                        engine=inst.engine,
                    )
                    new.append(nop)
                inst.sync_info = mybir.SyncInfo(
                    on_wait=keep, on_update=list(si.on_update)
                )
            new.append(inst)
        ordered[bbname] = new
    print(f"[waitsplit] inserted {n[0]} carrier nops")
    return _orig_lower_ordered(self, ordered)


tile.TileContext._lower_ordered_insts = _split_lower

_PROGRAM = None



def _build_program():
    nc = bass.Bass()
    xT_e = nc.declare_dram_parameter("xT", [DIM, N], FP32, isOutput=False)
    wq_e = nc.declare_dram_parameter("wq", [DIM, NH * DH], FP32, isOutput=False)
    wkv_e = nc.declare_dram_parameter("wkv", [DIM, 2 * DH], FP32, isOutput=False)
    wo_e = nc.declare_dram_parameter("wo", [NH * DH, DIM], FP32, isOutput=False)
    mk_e = nc.declare_dram_parameter("mk", [NH, N, KJD], FP32, isOutput=False)
    mv_e = nc.declare_dram_parameter("mv", [NH, N, KJD], FP32, isOutput=False)
    nk_e = nc.declare_dram_parameter("nk", [128, DH], FP32, isOutput=False)
    nv_e = nc.declare_dram_parameter("nv", [128, DH], FP32, isOutput=False)
    gg_e = nc.declare_dram_parameter("gg", [128, 4], FP32, isOutput=False)
    mask_e = nc.declare_dram_parameter("mask", [128, 128], FP32, isOutput=False)
    iden_e = nc.declare_dram_parameter("iden", [128, 128], FP32, isOutput=False)
    out_e = nc.declare_dram_parameter("out", [N, DIM], FP32, isOutput=True)

    AX = mybir.AxisListType.X
    EXP = mybir.ActivationFunctionType.Exp
    MULT = mybir.AluOpType.mult

    with tile.TileContext(nc) as tc:
        with tc.tile_pool(name="persist", bufs=1) as pp:
            qTh = [pp.tile([64, N], FP32, tag=f"qT{h}", name=f"qT{h}") for h in range(NH)]
            kT = pp.tile([64, N], FP32)
            vT = pp.tile([64, N], FP32)
            qnat = pp.tile([128, NT * 128], FP32) # per qi: [h0 d64 | h1 d64]
            vone = pp.tile([128, NT * 65], FP32)  # per ki tile: [v | 1]
            wo_sb = pp.tile([128, DIM], FP32)
            nk_sb = pp.tile([128, DH], FP32)
            nv_sb = pp.tile([128, DH], FP32)
            gg_sb = pp.tile([128, 4], FP32)
            mask_sb = pp.tile([128, 128], FP32)
            iden_sb = pp.tile([128, 128], FP32)
            nc.sync.dma_start(wo_sb[:], wo_e[:])
            nc.sync.dma_start(nk_sb[:], nk_e[:])
            nc.sync.dma_start(nv_sb[:], nv_e[:])
            nc.sync.dma_start(gg_sb[:], gg_e[:])
            nc.sync.dma_start(mask_sb[:], mask_e[:])
            nc.sync.dma_start(iden_sb[:], iden_e[:])

            # ---------------- stage A: projections + transposes ----------------
            with tc.tile_pool(name="stageA", bufs=2) as pa, \
                 tc.tile_pool(name="psA", bufs=2, space="PSUM") as psA:
                w_sb = pa.tile([128, 8 * 256], FP32, tag="w")
                # load both weight mats: per d-chunk 128x(128+128)
                for d in range(8):
                    nc.sync.dma_start(
                        w_sb[:, d * 256 : d * 256 + 128],
                        wq_e[d * 128 : (d + 1) * 128, :],
                    )
                    nc.sync.dma_start(
                        w_sb[:, d * 256 + 128 : d * 256 + 256],
                        wkv_e[d * 128 : (d + 1) * 128, :],
                    )
                xt_tiles = []
                for d in range(8):
                    xt = pa.tile([128, N], FP32, tag=f"xt{d}")
                    nc.sync.dma_start(xt[:], xT_e[d * 128 : (d + 1) * 128, :])
                    xt_tiles.append(xt)
                for nb in range(4):
                    sl = slice(nb * 512, (nb + 1) * 512)
                    targets = [
                        (qTh[0], 0), (qTh[1], 64), (kT, 128), (vT, 192),
                    ]
                    for dst, woff in targets:
                        ps = psA.tile([64, 512], FP32, tag="mm")
                        for d in range(8):
                            nc.tensor.matmul(
                                ps[:],
                                w_sb[:, d * 256 + woff : d * 256 + woff + 64],
                                xt_tiles[d][:, sl],
                                start=(d == 0),
                                stop=(d == 7),
                            )
                        nc.scalar.copy(dst[:, sl], ps[:])
                # q_nat: transpose each head's 64x128 qT block
                for qi in range(NT):
                    for h in range(NH):
                        tp = psA.tile([128, 64], FP32, tag="tp")
                        nc.tensor.transpose(
                            tp[:],
                            qTh[h][:, qi * 128 : (qi + 1) * 128],
                            iden_sb[0:64, 0:64],
                        )
                        nc.scalar.copy(
                            qnat[:, qi * 128 + h * 64 : qi * 128 + (h + 1) * 64],
                            tp[:],
                        )
                # v_nat (+ ones col)
                for ki in range(NT):
                    tp2 = psA.tile([128, 64], FP32, tag="tp2")
                    nc.tensor.transpose(
                        tp2[:],
                        vT[:, ki * 128 : (ki + 1) * 128],
                        iden_sb[0:64, 0:64],
                    )
                    nc.scalar.copy(vone[:, ki * 65 : ki * 65 + 64], tp2[:])
                    nc.vector.memset(vone[:, ki * 65 + 64 : ki * 65 + 65], 1.0)

            # ---------------- main loop ----------------
            with tc.tile_pool(name="mem", bufs=3) as pm, \
                 tc.tile_pool(name="small", bufs=4) as psm, \
                 tc.tile_pool(name="ps_st", bufs=2, space="PSUM") as ps_st, \
                 tc.tile_pool(name="ps_pv", bufs=2, space="PSUM") as ps_pv, \
                 tc.tile_pool(name="ps_c", bufs=1, space="PSUM") as ps_c:
                for qi in range(NT):
                    o2 = psm.tile([128, 128], FP32, tag="o2")
                    for h in range(NH):
                        qTh_ap = qTh[h][:, qi * 128 : (qi + 1) * 128]
                        qnh = qnat[:, qi * 128 + h * 64 : qi * 128 + h * 64 + 64]
                        # ---- local causal attention (transposed-scores flash) ----
                        pv = ps_pv.tile([128, 65], FP32, tag="pv")
                        for ki in range(qi + 1):
                            st = ps_st.tile([128, 128], FP32, tag="st")
                            nc.tensor.matmul(
                                st[:],
                                kT[:, ki * 128 : (ki + 1) * 128],
                                qTh_ap,
                                start=True,
                                stop=True,
                            )
                            if ki == qi:
                                nc.vector.tensor_add(st[:], st[:], mask_sb[:])
                            pt = psm.tile([128, 128], FP32, tag="pt")
                            nc.scalar.activation(pt[:], st[:], EXP, scale=SCALE)
                            nc.tensor.matmul(
                                pv[:],
                                pt[:],
                                vone[:, ki * 65 : ki * 65 + 65],
                                start=(ki == 0),
                                stop=(ki == qi),
                            )
                        # ---- memory branch ----
                        mk_t = pm.tile([128, KJD], FP32, tag="mk")
                        nc.sync.dma_start(mk_t[:], mk_e[h, qi * 128 : (qi + 1) * 128, :])
                        mv_t = pm.tile([128, KJD], FP32, tag="mv")
                        nc.sync.dma_start(mv_t[:], mv_e[h, qi * 128 : (qi + 1) * 128, :])

                        sim33 = psm.tile([128, 33], FP32, tag="sim")
                        # null-key sim -> col 0
                        nscr = psm.tile([128, DH], FP32, tag="nscr")
                        nc.vector.tensor_mul(nscr[:], qnh, nk_sb[:])
                        nc.vector.reduce_sum(sim33[:, 0:1], nscr[:], axis=AX)
                        # mem keys sim -> cols 1..32
                        prod = pm.tile([128, KJD], FP32, tag="prod")
                        q_bc = qnh.unsqueeze(1).broadcast_to([128, K, DH])
                        mk3 = mk_t[:].rearrange("p (j d) -> p j d", j=K)
                        nc.vector.tensor_mul(
                            prod[:].rearrange("p (j d) -> p j d", j=K), mk3, q_bc
                        )
                        nc.vector.reduce_sum(
                            sim33[:, 1:33],
                            prod[:].rearrange("p (j d) -> p j d", j=K),
                            axis=AX,
                        )
                        # softmax (no-max; logits are small by construction)
                        p33 = psm.tile([128, 33], FP32, tag="p33")
                        msum = psm.tile([128, 1], FP32, tag="msum")
                        nc.scalar.activation(
                            p33[:], sim33[:], EXP, scale=SCALE, accum_out=msum[:]
                        )
                        # weighted values: sum_j p_j * mv_j  (d-major product)
                        prod2 = pm.tile([128, KJD], FP32, tag="prod2")
                        mv3t = mv_t[:].rearrange("p (j d) -> p d j", j=K)
                        p_bc = p33[:, 1:33].unsqueeze(1).broadcast_to([128, DH, K])
                        pr2v = prod2[:].rearrange("p (d j) -> p d j", j=K)
                        nc.vector.tensor_mul(pr2v, mv3t, p_bc)
                        memv = psm.tile([128, DH], FP32, tag="memv")
                        nc.vector.reduce_sum(memv[:], pr2v, axis=AX)
                        # + null value
                        nvp = psm.tile([128, DH], FP32, tag="nvp")
                        nc.vector.tensor_scalar(
                            nvp[:], nv_sb[:], p33[:, 0:1], None, op0=MULT
                        )
                        nc.vector.tensor_add(memv[:], memv[:], nvp[:])
                        # ---- combine ----
                        pv_sb = psm.tile([128, 65], FP32, tag="pvsb")
                        nc.scalar.copy(pv_sb[:], pv[:])
                        linv = psm.tile([128, 1], FP32, tag="linv")
                        nc.vector.reciprocal(linv[:], pv_sb[:, 64:65])
                        lg = psm.tile([128, 1], FP32, tag="lg")
                        nc.scalar.mul(lg[:], linv[:], gg_sb[:, h : h + 1])
                        minv = psm.tile([128, 1], FP32, tag="minv")
                        nc.vector.reciprocal(minv[:], msum[:])
                        mg = psm.tile([128, 1], FP32, tag="mg")
                        nc.scalar.mul(mg[:], minv[:], gg_sb[:, 2 + h : 3 + h])
                        osl = o2[:, h * 64 : (h + 1) * 64]
                        nc.vector.tensor_scalar(
                            osl, pv_sb[:, 0:64], lg[:], None, op0=MULT
                        )
                        tmp = psm.tile([128, DH], FP32, tag="tmp")
                        nc.vector.tensor_scalar(
                            tmp[:], memv[:], mg[:], None, op0=MULT
                        )
                        nc.vector.tensor_add(osl, osl, tmp[:])
                    # ---- output projection for this qi ----
                    otp = ps_c.tile([128, 128], FP32, tag="otp")
                    nc.tensor.transpose(otp[:], o2[:], iden_sb[:])
                    ot_sb = psm.tile([128, 128], FP32, tag="otsb")
                    nc.scalar.copy(ot_sb[:], otp[:])
                    op_ps = ps_c.tile([128, DIM], FP32, tag="ops")
                    for half in range(2):
                        nc.tensor.matmul(
                            op_ps[:, half * 512 : (half + 1) * 512],
                            ot_sb[:],
                            wo_sb[:, half * 512 : (half + 1) * 512],
                            start=True,
                            stop=True,
                        )
                    out_sb = psm.tile([128, DIM], FP32, tag="outsb")
                    nc.scalar.copy(out_sb[:, 0:512], op_ps[:, 0:512])
                    nc.vector.tensor_copy(out_sb[:, 512:1024], op_ps[:, 512:1024])
                    nc.sync.dma_start(
                        out_e[qi * 128 : (qi + 1) * 128, :], out_sb[:]
                    )
    return nc


def _get_program():
    global _PROGRAM
    if _PROGRAM is None:
        _PROGRAM = _build_program()
    return _PROGRAM


def kernel(x, Wq, Wkv, Wo, bo, null_k, null_v, gate, mem_kv, mem_mask):
    x = np.asarray(x, dtype=np.float32)
    Wq = np.asarray(Wq, dtype=np.float32)
    Wkv = np.asarray(Wkv, dtype=np.float32)
    Wo = np.asarray(Wo, dtype=np.float32)
    bo = np.asarray(bo, dtype=np.float32)
    null_k = np.asarray(null_k, dtype=np.float32)
    null_v = np.asarray(null_v, dtype=np.float32)
    gate = np.asarray(gate, dtype=np.float32)
    mem_kv = np.asarray(mem_kv, dtype=np.float32)

    nc = _get_program()
    g = 1.0 / (1.0 + np.exp(-gate.reshape(H)))  # sigmoid, per head
    mem6 = mem_kv.reshape(B, H, N, K, 2, DH)
    iden = np.eye(128, dtype=np.float32)
    maskT = np.where(
        np.arange(128)[:, None] <= np.arange(128)[None, :], 0.0, NEG
    ).astype(np.float32)
    nk_rep = np.tile(null_k[None, :], (128, 1)).astype(np.float32)
    nv_rep = np.tile(null_v[None, :], (128, 1)).astype(np.float32)

    in_maps = []
    for c in range(8):
        b, hg = c // 4, c % 4
        h0 = hg * NH
        xT = np.ascontiguousarray(x[b].T)
        wq_c = np.ascontiguousarray(Wq[:, h0 * DH : (h0 + NH) * DH])
        wo_c = np.ascontiguousarray(Wo[h0 * DH : (h0 + NH) * DH, :])
        mk_c = np.ascontiguousarray(
            mem6[b, h0 : h0 + NH, :, :, 0, :].reshape(NH, N, KJD)
        )
        mv_c = np.ascontiguousarray(
            mem6[b, h0 : h0 + NH, :, :, 1, :].reshape(NH, N, KJD)
        )
        gg = np.zeros((128, 4), dtype=np.float32)
        gg[:, 0] = g[h0]
        gg[:, 1] = g[h0 + 1]
        gg[:, 2] = 1.0 - g[h0]
        gg[:, 3] = 1.0 - g[h0 + 1]
        in_maps.append(
            dict(
                xT=xT, wq=wq_c, wkv=Wkv, wo=wo_c, mk=mk_c, mv=mv_c,
                nk=nk_rep, nv=nv_rep, gg=gg, mask=maskT, iden=iden,
            )
        )

    global _last_in_maps
    _last_in_maps = in_maps
    res = run_bass_kernel_spmd(nc, in_maps, list(range(8)))
    out = np.zeros((B, N, DIM), dtype=np.float32)
    for c in range(8):
        out[c // 4] += res.results[c]["out"]
    out += bo[None, None, :]
    return out



# revision 10
# speedup vs baseline: 1.7758x; 1.7758x over previous
import sys
import os

sys.path.insert(0, "/opt/trn_rl_repo")

import numpy as np
import ml_dtypes

import concourse.bass as bass
import concourse.tile as tile
from concourse import mybir
from concourse.bass_utils import run_bass_kernel_spmd

# ---------------- problem constants (hardcoded) ----------------
B, N, DIM, H, DH, K = 2, 2048, 1024, 8, 64, 32
INNER = H * DH          # 512
NH = 2                  # heads per core
NT = N // 128           # 16 query/key tiles
KJD = K * DH            # 2048 mem-key (or value) floats per query
K1 = K + 1              # 33 keys incl. null slot
KD1 = K1 * DH           # 2112
SCALE = DH ** -0.5
NEG = -3.0e38

FP32 = mybir.dt.float32
BF16 = mybir.dt.bfloat16
BF = ml_dtypes.bfloat16


# ---------------- drain workaround (this walrus rejects multi-wait Drain) ---
def _patched_drain(self, tick_clock, wait_clock):
    nc = self.nc
    drain_inst = nc.sync.drain()
    from concourse.tile import ScopedClock

    wait_clock.add_sem_waits(
        drain_inst.ins, ScopedClock({None: tick_clock.global_clock})
    )
    si = drain_inst.ins.sync_info
    waits = list(si.on_wait)
    if len(waits) > 1:
        drain_inst.ins.sync_info = type(si)(on_wait=waits[:1], on_update=[])
        for w in waits[1:]:
            nop = nc.sync.nop(nofuse=True)
            nop.ins.sync_info = type(si)(on_wait=[w], on_update=[])
    nc.all_engine_barrier()
    popped = nc._tile_sem_poison_stack.pop()
    assert popped is self._sem_poison
    nc.clear_and_free_semaphores(list(self.sems.allocated().values()))
    nc.all_engine_barrier()


tile.TileContext._drain_and_barrier = _patched_drain


# ---- split multi-wait instructions (walrus wait-slot limit) ----
_MAXW = 1
_orig_lower_ordered = tile.TileContext._lower_ordered_insts


def _split_lower(self, ordered):
    n = [0]
    for bbname in list(ordered.keys()):
        insts = ordered[bbname]
        new = []
        for inst in insts:
            try:
                si = inst.sync_info
                waits = list(si.on_wait) if si is not None else []
            except AttributeError:
                waits = []
            if len(waits) > _MAXW:
                keep = waits[-_MAXW:]
                extra = waits[:-_MAXW]
                for i in range(0, len(extra), _MAXW):
                    chunk = extra[i : i + _MAXW]
                    n[0] += 1
                    nop = mybir.InstNoOp(
                        name=f"waitnop-{n[0]}-{inst.name}",
                        sync_info=mybir.SyncInfo(on_wait=chunk, on_update=[]),
                        bass_nofuse=True,
                        engine=inst.engine,
                    )
                    new.append(nop)
                inst.sync_info = mybir.SyncInfo(
                    on_wait=keep, on_update=list(si.on_update)
                )
            new.append(inst)
        ordered[bbname] = new
    print(f"[waitsplit] inserted {n[0]} carrier nops")
    return _orig_lower_ordered(self, ordered)


tile.TileContext._lower_ordered_insts = _split_lower

_PROGRAM = None


def _build_program():
    nc = bass.Bass()
    xT_e = nc.declare_dram_parameter("xT", [DIM, N], BF16, isOutput=False)
    wq_e = nc.declare_dram_parameter("wq", [DIM, NH * DH], BF16, isOutput=False)
    wkv_e = nc.declare_dram_parameter("wkv", [DIM, 2 * DH], BF16, isOutput=False)
    wo_e = nc.declare_dram_parameter("wo", [NH * DH, DIM], BF16, isOutput=False)
    # per (head, row): [ mk j-major 2048 | mv d-major 2048 ]
    mkv_e = nc.declare_dram_parameter("mkv", [NH, N, 2 * KD1], BF16, isOutput=False)
    gg_e = nc.declare_dram_parameter("gg", [128, 4], FP32, isOutput=False)
    mask_e = nc.declare_dram_parameter("mask", [128, 256], FP32, isOutput=False)
    iden_e = nc.declare_dram_parameter("iden", [128, 128], BF16, isOutput=False)
    out_e = nc.declare_dram_parameter("out", [N, DIM], BF16, isOutput=True)

    AX = mybir.AxisListType.X
    EXP = mybir.ActivationFunctionType.Exp
    MULT = mybir.AluOpType.mult
    ADD = mybir.AluOpType.add
    AVG = mybir.PoolFunctionType.avg

    with tile.TileContext(nc) as tc:
        with tc.tile_pool(name="persist", bufs=1) as pp:
            # qTcat: per tile ti, cols [ti*256 : ti*256+128] = head0 qT,
            # [+128:+256] = head1 qT  (d on partitions 0:64)
            qTcat = pp.tile([64, 2 * N], BF16)
            kvT = pp.tile([128, N], BF16)     # rows 0:64 kT, 64:128 vT
            qnat = pp.tile([128, NT * 128], BF16)  # per ti: h0 d64 | h1 d64
            vone = pp.tile([128, NT * 65], BF16)   # per ki tile: [v | 1]
            wo_sb = pp.tile([128, DIM], BF16)
            gg_sb = pp.tile([128, 4], FP32)
            mask_sb = pp.tile([128, 256], FP32)
            iden_sb = pp.tile([128, 128], BF16)
            nc.sync.dma_start(wo_sb[:], wo_e[:])
            nc.sync.dma_start(gg_sb[:], gg_e[:])
            nc.sync.dma_start(mask_sb[:], mask_e[:])
            nc.sync.dma_start(iden_sb[:], iden_e[:])

            # ---------------- stage A: projections + transposes ----------------
            with tc.tile_pool(name="stageA", bufs=2) as pa, \
                 tc.tile_pool(name="psA", bufs=2, space="PSUM") as psA:
                w_sb = pa.tile([128, 8 * 256], BF16, tag="w")
                # per d-chunk: wq (128 cols: h0|h1) then wkv (128 cols: k|v)
                for d in range(8):
                    nc.sync.dma_start(
                        w_sb[:, d * 256 : d * 256 + 128],
                        wq_e[d * 128 : (d + 1) * 128, :],
                    )
                    nc.sync.dma_start(
                        w_sb[:, d * 256 + 128 : d * 256 + 256],
                        wkv_e[d * 128 : (d + 1) * 128, :],
                    )
                xt_tiles = []
                for d in range(8):
                    xt = pa.tile([128, N], BF16, tag=f"xt{d}")
                    nc.sync.dma_start(xt[:], xT_e[d * 128 : (d + 1) * 128, :])
                    xt_tiles.append(xt)
                qTcat_v = qTcat[:].rearrange("p (t c) -> p t c", c=256)
                for nb in range(4):
                    sl = slice(nb * 512, (nb + 1) * 512)
                    # q head groups -> psum [64, 512] each
                    for h in range(NH):
                        psq = psA.tile([64, 512], FP32, tag="mmq")
                        for d in range(8):
                            nc.tensor.matmul(
                                psq[:],
                                w_sb[:, d * 256 + h * 64 : d * 256 + h * 64 + 64],
                                xt_tiles[d][:, sl],
                                start=(d == 0),
                                stop=(d == 7),
                            )
                        dst = qTcat_v[:, nb * 4 : (nb + 1) * 4,
                                      h * 128 : (h + 1) * 128]
                        src = psq[:].rearrange("p (t c) -> p t c", c=128)
                        nc.scalar.copy(dst, src)
                    # kv group -> psum [128, 512]
                    pskv = psA.tile([128, 512], FP32, tag="mmkv")
                    for d in range(8):
                        nc.tensor.matmul(
                            pskv[:],
                            w_sb[:, d * 256 + 128 : d * 256 + 256],
                            xt_tiles[d][:, sl],
                            start=(d == 0),
                            stop=(d == 7),
                        )
                    nc.scalar.copy(kvT[:, sl], pskv[:])
                # q_nat: transpose each head's 64x128 qT block;
                # vone: transpose vT blocks (rows 64:128, identity block at
                # [64:128, 64:128])
                for ti in range(NT):
                    for h in range(NH):
                        tp = psA.tile([128, 64], BF16, tag="tp")
                        nc.tensor.transpose(
                            tp[:],
                            qTcat[:, ti * 256 + h * 128 : ti * 256 + (h + 1) * 128],
                            iden_sb[0:64, 0:64],
                        )
                        nc.scalar.copy(
                            qnat[:, ti * 128 + h * 64 : ti * 128 + (h + 1) * 64],
                            tp[:],
                        )
                    tv = psA.tile([128, 64], BF16, tag="tv")
                    nc.tensor.transpose(
                        tv[:],
                        kvT[64:128, ti * 128 : (ti + 1) * 128],
                        iden_sb[64:128, 64:128],
                    )
                    nc.scalar.copy(vone[:, ti * 65 : ti * 65 + 64], tv[:])
                    nc.vector.memset(vone[:, ti * 65 + 64 : ti * 65 + 65], 1.0)

            # ---------------- main loop ----------------
            with tc.tile_pool(name="mem", bufs=3) as pm, \
                 tc.tile_pool(name="small", bufs=4) as psm, \
                 tc.tile_pool(name="ps_st", bufs=2, space="PSUM") as ps_st, \
                 tc.tile_pool(name="ps_pv", bufs=2, space="PSUM") as ps_pv, \
                 tc.tile_pool(name="ps_c", bufs=1, space="PSUM") as ps_c:
                for qi in range(NT):
                    o2 = psm.tile([128, 128], BF16, tag="o2")
                    # ---- local causal attention, both heads per st matmul ----
                    pv_h0 = ps_pv.tile([128, 65], FP32, tag="pv0")
                    pv_h1 = ps_pv.tile([128, 65], FP32, tag="pv1")
                    pvs = [pv_h0[:], pv_h1[:]]
                    for ki in range(qi + 1):
                        st2 = ps_st.tile([128, 256], FP32, tag="st")
                        nc.tensor.matmul(
                            st2[:],
                            kvT[0:64, ki * 128 : (ki + 1) * 128],
                            qTcat[:, qi * 256 : (qi + 1) * 256],
                            start=True,
                            stop=True,
                        )
                        if ki == qi:
                            nc.vector.tensor_add(st2[:], st2[:], mask_sb[:])
                        pt2 = psm.tile([128, 256], BF16, tag="pt")
                        nc.scalar.activation(pt2[:], st2[:], EXP, scale=SCALE)
                        for h in range(NH):
                            nc.tensor.matmul(
                                pvs[h],
                                pt2[:, h * 128 : (h + 1) * 128],
                                vone[:, ki * 65 : ki * 65 + 65],
                                start=(ki == 0),
                                stop=(ki == qi),
                            )
                    # ---- memory branch ----
                    for h in range(NH):
                        qnh = qnat[:, qi * 128 + h * 64 : qi * 128 + h * 64 + 64]
                        mkv_t = pm.tile([128, 2 * KD1], BF16, tag="mkv")
                        nc.sync.dma_start(
                            mkv_t[:], mkv_e[h, qi * 128 : (qi + 1) * 128, :]
                        )
                        # key sims incl. null slot (j = 0)
                        sim33 = psm.tile([128, K1], FP32, tag="sim33")
                        prod_k = pm.tile([128, KD1], BF16, tag="prodk")
                        q_bc = qnh.unsqueeze(1).broadcast_to([128, K1, DH])
                        mk3 = mkv_t[:, 0:KD1].rearrange("p (j d) -> p j d", j=K1)
                        nc.vector.tensor_mul(
                            prod_k[:].rearrange("p (j d) -> p j d", j=K1), mk3, q_bc
                        )
                        nc.vector.reduce_sum(
                            sim33[:],
                            prod_k[:].rearrange("p (j d) -> p j d", j=K1),
                            axis=AX,
                        )
                        # softmax (no-max; logits are small by construction)
                        p33 = psm.tile([128, K1], BF16, tag="p33")
                        msum = psm.tile([128, 1], FP32, tag="msum")
                        nc.scalar.activation(
                            p33[:], sim33[:], EXP, scale=SCALE, accum_out=msum[:]
                        )
                        # weighted values: d-major mv (incl. null col), mul on
                        # gpsimd, reduce on DVE
                        prod_v = pm.tile([128, KD1], BF16, tag="prodv")
                        mv3 = mkv_t[:, KD1 : 2 * KD1].rearrange(
                            "p (d j) -> p d j", j=K1
                        )
                        p_bc = p33[:].unsqueeze(1).broadcast_to([128, DH, K1])
                        pr3 = prod_v[:].rearrange("p (d j) -> p d j", j=K1)
                        nc.gpsimd.tensor_mul(pr3, mv3, p_bc)
                        memv = psm.tile([128, DH], FP32, tag="memv")
                        nc.vector.reduce_sum(memv[:], pr3, axis=AX)
                        # ---- combine ----
                        pv_sb = psm.tile([128, 65], FP32, tag="pvsb")
                        nc.scalar.copy(pv_sb[:], pvs[h])
                        linv = psm.tile([128, 1], FP32, tag="linv")
                        nc.vector.reciprocal(linv[:], pv_sb[:, 64:65])
                        lg = psm.tile([128, 1], FP32, tag="lg")
                        nc.scalar.mul(lg[:], linv[:], gg_sb[:, h : h + 1])
                        minv = psm.tile([128, 1], FP32, tag="minv")
                        nc.vector.reciprocal(minv[:], msum[:])
                        mg = psm.tile([128, 1], FP32, tag="mg")
                        nc.scalar.mul(mg[:], minv[:], gg_sb[:, 2 + h : 3 + h])
                        osl = o2[:, h * 64 : (h + 1) * 64]
                        nc.vector.tensor_scalar(
                            osl, pv_sb[:, 0:64], lg[:], None, op0=MULT
                        )
                        nc.vector.scalar_tensor_tensor(
                            osl, memv[:], mg[:], osl, op0=MULT, op1=ADD
                        )
                    # ---- output projection for this qi ----
                    otp = ps_c.tile([128, 128], BF16, tag="otp")
                    nc.tensor.transpose(otp[:], o2[:], iden_sb[:])
                    ot_sb = psm.tile([128, 128], BF16, tag="otsb")
                    nc.scalar.copy(ot_sb[:], otp[:])
                    out_sb = psm.tile([128, DIM], BF16, tag="outsb")
                    for half in range(2):
                        op_ps = ps_c.tile([128, 512], FP32, tag="ops")
                        nc.tensor.matmul(
                            op_ps[:],
                            ot_sb[:],
                            wo_sb[:, half * 512 : (half + 1) * 512],
                            start=True,
                            stop=True,
                        )
                        if half == 0:
                            nc.scalar.copy(out_sb[:, 0:512], op_ps[:])
                        else:
                            nc.vector.tensor_copy(out_sb[:, 512:1024], op_ps[:])
                    nc.sync.dma_start(
                        out_e[qi * 128 : (qi + 1) * 128, :], out_sb[:]
                    )
    return nc


def _get_program():
    global _PROGRAM
    if _PROGRAM is None:
        _PROGRAM = _build_program()
    return _PROGRAM


def _bf(a):
    return np.ascontiguousarray(a).astype(BF)


def kernel(x, Wq, Wkv, Wo, bo, null_k, null_v, gate, mem_kv, mem_mask):
    x = np.asarray(x, dtype=np.float32)
    Wq = np.asarray(Wq, dtype=np.float32)
    Wkv = np.asarray(Wkv, dtype=np.float32)
    Wo = np.asarray(Wo, dtype=np.float32)
    bo = np.asarray(bo, dtype=np.float32)
    null_k = np.asarray(null_k, dtype=np.float32)
    null_v = np.asarray(null_v, dtype=np.float32)
    gate = np.asarray(gate, dtype=np.float32)
    mem_kv = np.asarray(mem_kv, dtype=np.float32)

    nc = _get_program()
    g = 1.0 / (1.0 + np.exp(-gate.reshape(H)))  # sigmoid, per head
    mem_bf = mem_kv.astype(BF)
    mem6 = mem_bf.reshape(B, H, N, K, 2, DH)
    iden = np.eye(128, dtype=BF)
    maskT = np.where(
        np.arange(128)[:, None] <= np.arange(128)[None, :], 0.0, NEG
    ).astype(np.float32)
    mask2 = np.concatenate([maskT, maskT], axis=1)  # [128, 256]
    nk_bf = null_k.astype(BF)
    nv_bf = null_v.astype(BF)

    in_maps = []
    for c in range(8):
        b, hg = c // 4, c % 4
        h0 = hg * NH
        xT = _bf(x[b].T)
        wq_c = _bf(Wq[:, h0 * DH : (h0 + NH) * DH])
        wo_c = _bf(Wo[h0 * DH : (h0 + NH) * DH, :])
        mkv_c = np.empty((NH, N, 2 * KD1), dtype=BF)
        for hh in range(NH):
            # keys j-major, null slot at j=0
            mkc = mkv_c[hh, :, :KD1].reshape(N, K1, DH)
            mkc[:, 0, :] = nk_bf[None, :]
            mkc[:, 1:, :] = mem6[b, h0 + hh, :, :, 0, :]
            # values d-major, null col at j=0
            mvc = mkv_c[hh, :, KD1:].reshape(N, DH, K1)
            mvc[:, :, 0] = nv_bf[None, :]
            mvc[:, :, 1:] = mem6[b, h0 + hh, :, :, 1, :].transpose(0, 2, 1)
        gg = np.zeros((128, 4), dtype=np.float32)
        gg[:, 0] = g[h0]
        gg[:, 1] = g[h0 + 1]
        gg[:, 2] = 1.0 - g[h0]
        gg[:, 3] = 1.0 - g[h0 + 1]
        in_maps.append(
            dict(
                xT=xT, wq=wq_c, wkv=_bf(Wkv), wo=wo_c, mkv=mkv_c,
                gg=gg, mask=mask2, iden=iden,
            )
        )

    global _last_in_maps
    _last_in_maps = in_maps
    res = run_bass_kernel_spmd(nc, in_maps, list(range(8)))
    out = np.zeros((B, N, DIM), dtype=np.float32)
    for c in range(8):
        out[c // 4] += res.results[c]["out"].astype(np.float32)
    out += bo[None, None, :]
    return out


# revision 11
# speedup vs baseline: 1.9173x; 1.0797x over previous
import sys
import os

sys.path.insert(0, "/opt/trn_rl_repo")

import numpy as np
import ml_dtypes

import concourse.bass as bass
import concourse.tile as tile
from concourse import mybir
from concourse.bass_utils import run_bass_kernel_spmd

# ---------------- problem constants (hardcoded) ----------------
B, N, DIM, H, DH, K = 2, 2048, 1024, 8, 64, 32
INNER = H * DH          # 512
NH = 2                  # heads per core
NT = N // 128           # 16 query/key tiles
KJD = K * DH            # 2048 mem-key (or value) floats per query
K1 = K + 1              # 33 keys incl. null slot
KD1 = K1 * DH           # 2112
SCALE = DH ** -0.5
NEG = -3.0e38

FP32 = mybir.dt.float32
BF16 = mybir.dt.bfloat16
BF = ml_dtypes.bfloat16


# ---------------- drain workaround (this walrus rejects multi-wait Drain) ---
def _patched_drain(self, tick_clock, wait_clock):
    nc = self.nc
    drain_inst = nc.sync.drain()
    from concourse.tile import ScopedClock

    wait_clock.add_sem_waits(
        drain_inst.ins, ScopedClock({None: tick_clock.global_clock})
    )
    si = drain_inst.ins.sync_info
    waits = list(si.on_wait)
    if len(waits) > 1:
        drain_inst.ins.sync_info = type(si)(on_wait=waits[:1], on_update=[])
        for w in waits[1:]:
            nop = nc.sync.nop(nofuse=True)
            nop.ins.sync_info = type(si)(on_wait=[w], on_update=[])
    nc.all_engine_barrier()
    popped = nc._tile_sem_poison_stack.pop()
    assert popped is self._sem_poison
    nc.clear_and_free_semaphores(list(self.sems.allocated().values()))
    nc.all_engine_barrier()


tile.TileContext._drain_and_barrier = _patched_drain


# ---- split multi-wait instructions (walrus wait-slot limit) ----
_MAXW = 1
_orig_lower_ordered = tile.TileContext._lower_ordered_insts


def _split_lower(self, ordered):
    n = [0]
    for bbname in list(ordered.keys()):
        insts = ordered[bbname]
        new = []
        for inst in insts:
            try:
                si = inst.sync_info
                waits = list(si.on_wait) if si is not None else []
            except AttributeError:
                waits = []
            if len(waits) > _MAXW:
                keep = waits[-_MAXW:]
                extra = waits[:-_MAXW]
                for i in range(0, len(extra), _MAXW):
                    chunk = extra[i : i + _MAXW]
                    n[0] += 1
                    nop = mybir.InstNoOp(
                        name=f"waitnop-{n[0]}-{inst.name}",
                        sync_info=mybir.SyncInfo(on_wait=chunk, on_update=[]),
                        bass_nofuse=True,
                        engine=inst.engine,
                    )
                    new.append(nop)
                inst.sync_info = mybir.SyncInfo(
                    on_wait=keep, on_update=list(si.on_update)
                )
            new.append(inst)
        ordered[bbname] = new
    print(f"[waitsplit] inserted {n[0]} carrier nops")
    return _orig_lower_ordered(self, ordered)


tile.TileContext._lower_ordered_insts = _split_lower

_PROGRAM = None


def _build_program():
    nc = bass.Bass()
    xT_e = nc.declare_dram_parameter("xT", [DIM, N], BF16, isOutput=False)
    wq_e = nc.declare_dram_parameter("wq", [DIM, NH * DH], BF16, isOutput=False)
    wkv_e = nc.declare_dram_parameter("wkv", [DIM, 2 * DH], BF16, isOutput=False)
    wo_e = nc.declare_dram_parameter("wo", [NH * DH, DIM], BF16, isOutput=False)
    # per (head, row): [ mk j-major 2048 | mv d-major 2048 ]
    mkv_e = nc.declare_dram_parameter("mkv", [NH, N, 2 * KD1], BF16, isOutput=False)
    gg_e = nc.declare_dram_parameter("gg", [128, 4], FP32, isOutput=False)
    mask_e = nc.declare_dram_parameter("mask", [128, 256], FP32, isOutput=False)
    iden_e = nc.declare_dram_parameter("iden", [128, 128], BF16, isOutput=False)
    out_e = nc.declare_dram_parameter("out", [N, DIM], BF16, isOutput=True)

    AX = mybir.AxisListType.X
    EXP = mybir.ActivationFunctionType.Exp
    MULT = mybir.AluOpType.mult
    ADD = mybir.AluOpType.add
    AVG = mybir.PoolFunctionType.avg

    with tile.TileContext(nc) as tc:
        with tc.tile_pool(name="persist", bufs=1) as pp:
            # qTcat: per tile ti, cols [ti*256 : ti*256+128] = head0 qT,
            # [+128:+256] = head1 qT  (d on partitions 0:64)
            qTcat = pp.tile([64, 2 * N], BF16)
            kvT = pp.tile([128, N], BF16)     # rows 0:64 kT, 64:128 vT
            qnat = pp.tile([128, NT * 128], BF16)  # per ti: h0 d64 | h1 d64
            vone = pp.tile([128, NT * 65], BF16)   # per ki tile: [v | 1]
            wo_sb = pp.tile([128, DIM], BF16)
            gg_sb = pp.tile([128, 4], FP32)
            mask_sb = pp.tile([128, 256], FP32)
            iden_sb = pp.tile([128, 128], BF16)
            nc.sync.dma_start(wo_sb[:], wo_e[:])
            nc.sync.dma_start(gg_sb[:], gg_e[:])
            nc.sync.dma_start(mask_sb[:], mask_e[:])
            nc.sync.dma_start(iden_sb[:], iden_e[:])

            # ---------------- stage A: projections + transposes ----------------
            with tc.tile_pool(name="stageA", bufs=2) as pa, \
                 tc.tile_pool(name="psA", bufs=2, space="PSUM") as psA:
                w_sb = pa.tile([128, 8 * 256], BF16, tag="w")
                # per d-chunk: wq (128 cols: h0|h1) then wkv (128 cols: k|v)
                for d in range(8):
                    nc.sync.dma_start(
                        w_sb[:, d * 256 : d * 256 + 128],
                        wq_e[d * 128 : (d + 1) * 128, :],
                    )
                    nc.sync.dma_start(
                        w_sb[:, d * 256 + 128 : d * 256 + 256],
                        wkv_e[d * 128 : (d + 1) * 128, :],
                    )
                xt_tiles = []
                for d in range(8):
                    xt = pa.tile([128, N], BF16, tag=f"xt{d}")
                    nc.sync.dma_start(xt[:], xT_e[d * 128 : (d + 1) * 128, :])
                    xt_tiles.append(xt)
                qTcat_v = qTcat[:].rearrange("p (t c) -> p t c", c=256)
                for nb in range(4):
                    sl = slice(nb * 512, (nb + 1) * 512)
                    # q head groups -> psum [64, 512] each
                    for h in range(NH):
                        psq = psA.tile([64, 512], FP32, tag="mmq")
                        for d in range(8):
                            nc.tensor.matmul(
                                psq[:],
                                w_sb[:, d * 256 + h * 64 : d * 256 + h * 64 + 64],
                                xt_tiles[d][:, sl],
                                start=(d == 0),
                                stop=(d == 7),
                            )
                        dst = qTcat_v[:, nb * 4 : (nb + 1) * 4,
                                      h * 128 : (h + 1) * 128]
                        src = psq[:].rearrange("p (t c) -> p t c", c=128)
                        nc.scalar.copy(dst, src)
                    # kv group -> psum [128, 512]
                    pskv = psA.tile([128, 512], FP32, tag="mmkv")
                    for d in range(8):
                        nc.tensor.matmul(
                            pskv[:],
                            w_sb[:, d * 256 + 128 : d * 256 + 256],
                            xt_tiles[d][:, sl],
                            start=(d == 0),
                            stop=(d == 7),
                        )
                    nc.scalar.copy(kvT[:, sl], pskv[:])
                # q_nat: transpose each head's 64x128 qT block;
                # vone: transpose vT blocks (rows 64:128, identity block at
                # [64:128, 64:128])
                for ti in range(NT):
                    for h in range(NH):
                        tp = psA.tile([128, 64], BF16, tag="tp")
                        nc.tensor.transpose(
                            tp[:],
                            qTcat[:, ti * 256 + h * 128 : ti * 256 + (h + 1) * 128],
                            iden_sb[0:64, 0:64],
                        )
                        nc.scalar.copy(
                            qnat[:, ti * 128 + h * 64 : ti * 128 + (h + 1) * 64],
                            tp[:],
                        )
                    tv = psA.tile([128, 64], BF16, tag="tv")
                    nc.tensor.transpose(
                        tv[:],
                        kvT[64:128, ti * 128 : (ti + 1) * 128],
                        iden_sb[64:128, 64:128],
                    )
                    nc.scalar.copy(vone[:, ti * 65 : ti * 65 + 64], tv[:])
                    nc.vector.memset(vone[:, ti * 65 + 64 : ti * 65 + 65], 1.0)

            # ---------------- main loop ----------------
            with tc.tile_pool(name="mem", bufs=3) as pm, \
                 tc.tile_pool(name="small", bufs=4) as psm, \
                 tc.tile_pool(name="ps_st", bufs=2, space="PSUM") as ps_st, \
                 tc.tile_pool(name="ps_pv", bufs=2, space="PSUM") as ps_pv, \
                 tc.tile_pool(name="ps_c", bufs=1, space="PSUM") as ps_c:
                for qi in range(NT):
                    o2 = psm.tile([128, 128], BF16, tag="o2")
                    # ---- local causal attention, both heads per st matmul ----
                    pv_h0 = ps_pv.tile([128, 65], FP32, tag="pv0")
                    pv_h1 = ps_pv.tile([128, 65], FP32, tag="pv1")
                    pvs = [pv_h0[:], pv_h1[:]]
                    for ki in range(qi + 1):
                        st2 = ps_st.tile([128, 256], FP32, tag="st")
                        nc.tensor.matmul(
                            st2[:],
                            kvT[0:64, ki * 128 : (ki + 1) * 128],
                            qTcat[:, qi * 256 : (qi + 1) * 256],
                            start=True,
                            stop=True,
                        )
                        if ki == qi:
                            nc.vector.tensor_add(st2[:], st2[:], mask_sb[:])
                        pt2 = psm.tile([128, 256], BF16, tag="pt")
                        nc.scalar.activation(pt2[:], st2[:], EXP, scale=SCALE)
                        for h in range(NH):
                            nc.tensor.matmul(
                                pvs[h],
                                pt2[:, h * 128 : (h + 1) * 128],
                                vone[:, ki * 65 : ki * 65 + 65],
                                start=(ki == 0),
                                stop=(ki == qi),
                            )
                    # ---- memory branch ----
                    for h in range(NH):
                        qnh = qnat[:, qi * 128 + h * 64 : qi * 128 + h * 64 + 64]
                        mkv_t = pm.tile([128, 2 * KD1], BF16, tag="mkv")
                        nc.sync.dma_start(
                            mkv_t[:], mkv_e[h, qi * 128 : (qi + 1) * 128, :]
                        )
                        # key sims incl. null slot (j = 0)
                        sim33 = psm.tile([128, K1], FP32, tag="sim33")
                        prod_k = pm.tile([128, KD1], BF16, tag="prodk")
                        q_bc = qnh.unsqueeze(1).broadcast_to([128, K1, DH])
                        mk3 = mkv_t[:, 0:KD1].rearrange("p (j d) -> p j d", j=K1)
                        nc.vector.tensor_mul(
                            prod_k[:].rearrange("p (j d) -> p j d", j=K1), mk3, q_bc
                        )
                        nc.vector.reduce_sum(
                            sim33[:],
                            prod_k[:].rearrange("p (j d) -> p j d", j=K1),
                            axis=AX,
                        )
                        # softmax (no-max; logits are small by construction)
                        p33 = psm.tile([128, K1], BF16, tag="p33")
                        msum = psm.tile([128, 1], FP32, tag="msum")
                        nc.scalar.activation(
                            p33[:], sim33[:], EXP, scale=SCALE, accum_out=msum[:]
                        )
                        # weighted values: d-major mv (incl. null col), mul on
                        # gpsimd, reduce on DVE
                        prod_v = pm.tile([128, KD1], BF16, tag="prodv")
                        mv3 = mkv_t[:, KD1 : 2 * KD1].rearrange(
                            "p (d j) -> p d j", j=K1
                        )
                        p_bc = p33[:].unsqueeze(1).broadcast_to([128, DH, K1])
                        pr3 = prod_v[:].rearrange("p (d j) -> p d j", j=K1)
                        nc.gpsimd.tensor_mul(pr3, mv3, p_bc)
                        memv = psm.tile([128, DH], FP32, tag="memv")
                        nc.vector.reduce_sum(memv[:], pr3, axis=AX)
                        # ---- combine (ACT scales + GPSIMD add; DVE only recips) ----
                        pv_sb = psm.tile([128, 65], FP32, tag="pvsb")
                        nc.scalar.copy(pv_sb[:], pvs[h])
                        linv = psm.tile([128, 1], FP32, tag="linv")
                        nc.vector.reciprocal(linv[:], pv_sb[:, 64:65])
                        lg = psm.tile([128, 1], FP32, tag="lg")
                        nc.scalar.mul(lg[:], linv[:], gg_sb[:, h : h + 1])
                        minv = psm.tile([128, 1], FP32, tag="minv")
                        nc.vector.reciprocal(minv[:], msum[:])
                        mg = psm.tile([128, 1], FP32, tag="mg")
                        nc.scalar.mul(mg[:], minv[:], gg_sb[:, 2 + h : 3 + h])
                        oloc = psm.tile([128, DH], FP32, tag="oloc")
                        nc.scalar.mul(oloc[:], pv_sb[:, 0:64], lg[:])
                        omem = psm.tile([128, DH], FP32, tag="omem")
                        nc.scalar.mul(omem[:], memv[:], mg[:])
                        osl = o2[:, h * 64 : (h + 1) * 64]
                        nc.gpsimd.tensor_add(osl, oloc[:], omem[:])
                    # ---- output projection for this qi ----
                    otp = ps_c.tile([128, 128], BF16, tag="otp")
                    nc.tensor.transpose(otp[:], o2[:], iden_sb[:])
                    ot_sb = psm.tile([128, 128], BF16, tag="otsb")
                    nc.scalar.copy(ot_sb[:], otp[:])
                    out_sb = psm.tile([128, DIM], BF16, tag="outsb")
                    for half in range(2):
                        op_ps = ps_c.tile([128, 512], FP32, tag="ops")
                        nc.tensor.matmul(
                            op_ps[:],
                            ot_sb[:],
                            wo_sb[:, half * 512 : (half + 1) * 512],
                            start=True,
                            stop=True,
                        )
                        nc.scalar.copy(
                            out_sb[:, half * 512 : (half + 1) * 512], op_ps[:]
                        )
                    nc.sync.dma_start(
                        out_e[qi * 128 : (qi + 1) * 128, :], out_sb[:]
                    )
    return nc


def _get_program():
    global _PROGRAM
    if _PROGRAM is None:
        _PROGRAM = _build_program()
    return _PROGRAM


def _bf(a):
    return np.ascontiguousarray(a).astype(BF)


def kernel(x, Wq, Wkv, Wo, bo, null_k, null_v, gate, mem_kv, mem_mask):
    x = np.asarray(x, dtype=np.float32)
    Wq = np.asarray(Wq, dtype=np.float32)
    Wkv = np.asarray(Wkv, dtype=np.float32)
    Wo = np.asarray(Wo, dtype=np.float32)
    bo = np.asarray(bo, dtype=np.float32)
    null_k = np.asarray(null_k, dtype=np.float32)
    null_v = np.asarray(null_v, dtype=np.float32)
    gate = np.asarray(gate, dtype=np.float32)
    mem_kv = np.asarray(mem_kv, dtype=np.float32)

    nc = _get_program()
    g = 1.0 / (1.0 + np.exp(-gate.reshape(H)))  # sigmoid, per head
    mem_bf = mem_kv.astype(BF)
    mem6 = mem_bf.reshape(B, H, N, K, 2, DH)
    iden = np.eye(128, dtype=BF)
    maskT = np.where(
        np.arange(128)[:, None] <= np.arange(128)[None, :], 0.0, NEG
    ).astype(np.float32)
    mask2 = np.concatenate([maskT, maskT], axis=1)  # [128, 256]
    nk_bf = null_k.astype(BF)
    nv_bf = null_v.astype(BF)

    in_maps = []
    for c in range(8):
        b, hg = c // 4, c % 4
        h0 = hg * NH
        xT = _bf(x[b].T)
        wq_c = _bf(Wq[:, h0 * DH : (h0 + NH) * DH])
        wo_c = _bf(Wo[h0 * DH : (h0 + NH) * DH, :])
        mkv_c = np.empty((NH, N, 2 * KD1), dtype=BF)
        for hh in range(NH):
            # keys j-major, null slot at j=0
            mkc = mkv_c[hh, :, :KD1].reshape(N, K1, DH)
            mkc[:, 0, :] = nk_bf[None, :]
            mkc[:, 1:, :] = mem6[b, h0 + hh, :, :, 0, :]
            # values d-major, null col at j=0
            mvc = mkv_c[hh, :, KD1:].reshape(N, DH, K1)
            mvc[:, :, 0] = nv_bf[None, :]
            mvc[:, :, 1:] = mem6[b, h0 + hh, :, :, 1, :].transpose(0, 2, 1)
        gg = np.zeros((128, 4), dtype=np.float32)
        gg[:, 0] = g[h0]
        gg[:, 1] = g[h0 + 1]
        gg[:, 2] = 1.0 - g[h0]
        gg[:, 3] = 1.0 - g[h0 + 1]
        in_maps.append(
            dict(
                xT=xT, wq=wq_c, wkv=_bf(Wkv), wo=wo_c, mkv=mkv_c,
                gg=gg, mask=mask2, iden=iden,
            )
        )

    global _last_in_maps
    _last_in_maps = in_maps
    res = run_bass_kernel_spmd(nc, in_maps, list(range(8)))
    out = np.zeros((B, N, DIM), dtype=np.float32)
    for c in range(8):
        out[c // 4] += res.results[c]["out"].astype(np.float32)
    out += bo[None, None, :]
    return out
